# revision 28
# baseline (speedup 1.0000x reference)
"""Local-window (banded) multi-head attention on 8 Trainium2 NeuronCores.

Problem: x[L=2048, B=8, D=512], Wqkv[1536, 512], Wout[512, 512], bout[512].
  qkv = x @ Wqkv.T ; per-head banded attention (|i-j| <= 64, window 129);
  out = attn_out @ Wout.T + bout.

Sharding: batch B=8 across the 8 cores (data parallel). Each core runs the
full pipeline for one batch element. Inputs are pre-transposed host-side so
all device matmuls contract over the partition dimension:

  xT[d, l], WqkvT[d, c], WoutT[d', c] in SBUF; scores are computed
  TRANSPOSED (scoresT[m, l] = K @ Q^T) so that softmax normalization and
  the P@V contraction both happen along the partition (m) axis with zero
  on-chip transposes. The softmax denominator comes for free from an
  appended ones-column in V; normalization uses a tiny K=1 broadcast matmul.

Matmuls run in float32r (fp32 storage, fast PE path, N>=256).
"""

import os
import sys

import numpy as np

if "/opt/trn_rl_repo" not in sys.path:
    sys.path.insert(0, "/opt/trn_rl_repo")

L, B, D, H, DH = 2048, 8, 512, 8, 64
WIN, PAD = 129, 64
C3 = 3 * D  # 1536
NK = D // 128  # 4 contraction tiles
NLT = L // 128  # 16 l-tiles
NCH = L // 256  # 8 attention l-chunks of 256
HEAD_STRIDE = DH + 1  # 65: V columns per head incl. ones column

_NC_CACHE = {}


def _build_nc():
    from concourse import bacc, mybir, tile

    f32 = mybir.dt.float32
    f32r = mybir.dt.float32r
    Exp = mybir.ActivationFunctionType.Exp
    is_ge = mybir.AluOpType.is_ge

    nc = bacc.Bacc(None, target_bir_lowering=False)

    xT_d = nc.dram_tensor("xT", [D, L], f32r, kind="ExternalInput")
    wqkvT_d = nc.dram_tensor("wqkvT", [D, C3], f32r, kind="ExternalInput")
    woutT_d = nc.dram_tensor("woutT", [D, D], f32r, kind="ExternalInput")
    bout_d = nc.dram_tensor("bout", [D], f32, kind="ExternalInput")
    zeros_d = nc.dram_tensor("zeros_c", [128, 640], f32r, kind="ExternalInput")
    onesc_d = nc.dram_tensor("ones_c", [128, 8], f32r, kind="ExternalInput")
    y_d = nc.dram_tensor("y", [L, D], f32, kind="ExternalOutput")

    import concourse.bass as bass

    KTW = 64 + L + 64  # K^T cols: zero-pad both sides

    with tile.TileContext(nc) as tc, nc.allow_low_precision(
        reason="float32r tiles feed the PE fast path; accumulation stays fp32 in PSUM"
    ):
        with (
            tc.tile_pool(name="pers", bufs=1) as pers,
            tc.tile_pool(name="ps", bufs=1, space="PSUM") as ps,
        ):
            # ---- persistent SBUF tensors (everything stays resident) ----
            xT = [pers.tile([128, L], f32r, name=f"xT{k}", tag=f"xT{k}") for k in range(NK)]
            wqkvT = [
                pers.tile([128, C3], f32r, name=f"wqkvT{k}", tag=f"wqkvT{k}")
                for k in range(NK)
            ]
            woutT = [
                pers.tile([128, D], f32r, name=f"woutT{k}", tag=f"woutT{k}")
                for k in range(NK)
            ]
            boutb = pers.tile([128, D], f32, name="boutb", tag="boutb")
            ones1 = pers.tile([1, DH], f32r, name="ones1", tag="ones1")
            QT = [pers.tile([128, L], f32r, name=f"QT{t}", tag=f"QT{t}") for t in range(NK)]
            KT = [
                pers.tile([128, KTW], f32r, name=f"KT{t}", tag=f"KT{t}")
                for t in range(NK)
            ]
            Vs = [
                pers.tile([128, H * HEAD_STRIDE], f32r, name=f"Vs{j}", tag=f"Vs{j}")
                for j in range(NLT + 1)
            ]
            # per-chunk normalized O^T buffers (rotating, 2 l-tiles wide)
            OTc = [
                pers.tile([128, 256], f32r, name=f"OTc{t}", tag=f"OTc{t}", bufs=2)
                for t in range(NK)
            ]

            def mm(out, lhsT, rhs, start, stop):
                nc.tensor.matmul(out, lhsT, rhs, start=start, stop=stop)

            # ---- input DMAs: column-sliced + interleaved across both HWDGE
            # rings so the first projection groups unblock within ~3us ----
            for ch in range(4):
                cs = slice(ch * 512, (ch + 1) * 512)
                for k in range(NK):
                    eng = nc.sync if (k + ch) % 2 == 0 else nc.scalar
                    eng.dma_start(out=xT[k][:, cs], in_=xT_d[k * 128 : (k + 1) * 128, cs])
                # wqkvT thirds in V, Q, K priority order per round
                third = [2 * D, 0, D, None][ch]
                if third is not None:
                    ws = slice(third, third + 512)
                    for k in range(NK):
                        eng = nc.scalar if (k + ch) % 2 == 0 else nc.sync
                        eng.dma_start(
                            out=wqkvT[k][:, ws], in_=wqkvT_d[k * 128 : (k + 1) * 128, ws]
                        )
            for k in range(NK):
                nc.sync.dma_start(
                    out=woutT[k][:], in_=woutT_d[k * 128 : (k + 1) * 128, :]
                )
            bout_ap = bout_d[:]
            bout_bcast = bass.AP(
                tensor=bout_ap.tensor, offset=bout_ap.offset, ap=[[0, 128], [1, D]]
            )
            nc.gpsimd.dma_start(out=boutb[:], in_=bout_bcast)
            nc.gpsimd.dma_start(
                out=ones1[:], in_=onesc_d[0:DH, 0:1].rearrange("a b -> b a")
            )
            # zero K^T left pad and the out-of-range halves of the shifted V
            for t in range(NK):
                nc.sync.dma_start(out=KT[t][:, 0:64], in_=zeros_d[:, 0:64])
                nc.sync.dma_start(
                    out=KT[t][:, 64 + L : KTW], in_=zeros_d[:, 0:64]
                )
            nc.sync.dma_start(
                out=Vs[0][0:64, :], in_=zeros_d[0:64, 0 : H * HEAD_STRIDE]
            )
            nc.sync.dma_start(
                out=Vs[NLT][64:128, :], in_=zeros_d[0:64, 0 : H * HEAD_STRIDE]
            )
            # ones column for every head slot (softmax denom via PV matmul)
            for j in range(NLT + 1):
                vcol = Vs[j].rearrange("p (h e) -> p h e", e=HEAD_STRIDE)
                nc.gpsimd.dma_start(
                    out=vcol[:, :, DH : DH + 1],
                    in_=onesc_d[:].rearrange("p (h e) -> p h e", e=1),
                )

            # ---- phase B1: V projection (natural layout, shifted tiling) ----
            for lt in range(NLT):
                vp = ps.tile([128, D], f32, name=f"vp{lt}", tag="big", bufs=2)
                for k in range(NK):
                    mm(
                        vp[:],
                        xT[k][:, lt * 128 : (lt + 1) * 128],
                        wqkvT[k][:, 2 * D : 3 * D],
                        start=(k == 0),
                        stop=(k == NK - 1),
                    )
                src = vp.rearrange("p (h e) -> p h e", e=DH)
                dlo = Vs[lt][64:128, :].rearrange("p (h e) -> p h e", e=HEAD_STRIDE)
                dhi = Vs[lt + 1][0:64, :].rearrange("p (h e) -> p h e", e=HEAD_STRIDE)
                nc.scalar.copy(out=dlo[:, :, 0:DH], in_=src[0:64])
                nc.vector.tensor_copy(out=dhi[:, :, 0:DH], in_=src[64:128])

            # ---- phase B2: Q^T / K^T projections, l-chunk-major so attention
            # on early chunks can start while later chunks still project ----
            for ch in range(4):  # l-chunks of 512
                for t in range(NK):
                    for which in range(2):  # 0 -> Q tile t, 1 -> K tile t
                        c0 = which * D + t * 128
                        qp = ps.tile(
                            [128, 512], f32, name=f"qp{t}_{which}_{ch}",
                            tag="big", bufs=2,
                        )
                        for k in range(NK):
                            mm(
                                qp[:],
                                wqkvT[k][:, c0 : c0 + 128],
                                xT[k][:, ch * 512 : (ch + 1) * 512],
                                start=(k == 0),
                                stop=(k == NK - 1),
                            )
                        if which == 0:
                            dest = QT[t][:, ch * 512 : (ch + 1) * 512]
                        else:
                            dest = KT[t][:, 64 + ch * 512 : 64 + (ch + 1) * 512]
                        nc.vector.tensor_copy(out=dest, in_=qp[:])

            # ---- phase C+D: banded attention + fused output projection ----
            for ch in range(NCH):
                for t in range(NK):
                    otc = OTc[t] if True else None
                    otmp = None
                    for hh in range(2):
                        h = 2 * t + hh
                        p0 = hh * 64
                        qsl = QT[t][p0 : p0 + 64, ch * 256 : (ch + 1) * 256]
                        # fused scores psum: 3 m-tiles side by side (2 banks)
                        scp = ps.tile(
                            [128, 768], f32, name=f"sc{h}_{ch}", tag="sc", bufs=2
                        )
                        for r in range(3):
                            kcol = 256 * ch + 128 * r  # into padded KT columns
                            mm(
                                scp[:, 256 * r : 256 * (r + 1)],
                                KT[t][p0 : p0 + 64, kcol : kcol + 128],
                                qsl,
                                start=True,
                                stop=True,
                            )
                        pt = wk_tile = pers.tile(
                            [128, 768], f32r, name=f"pt{h}_{ch}", tag="pt", bufs=4
                        )
                        nc.scalar.activation(
                            out=pt[:], in_=scp[:], func=Exp, scale=0.125
                        )
                        # band mask per m-tile r: keep iff 0 <= (128r + p) - f <= 128
                        # fused as two 2-block selects over the 768-wide tile
                        pAB = pt[:, 0:512].rearrange("p (b f) -> p b f", f=256)
                        pBC = pt[:, 256:768].rearrange("p (b f) -> p b f", f=256)
                        nc.gpsimd.affine_select(
                            out=pAB, in_=pAB, compare_op=is_ge, fill=0.0,
                            base=0, pattern=[[128, 2], [-1, 256]],
                            channel_multiplier=1,
                        )
                        nc.gpsimd.affine_select(
                            out=pBC, in_=pBC, compare_op=is_ge, fill=0.0,
                            base=0, pattern=[[-128, 2], [1, 256]],
                            channel_multiplier=-1,
                        )
                        if ch == 0:  # global key index p-64 must be >= 0 (r0)
                            p_r0 = pt[:, 0:256]
                            nc.gpsimd.affine_select(
                                out=p_r0, in_=p_r0, compare_op=is_ge, fill=0.0,
                                base=-64, pattern=[[0, 256]], channel_multiplier=1,
                            )
                        if ch == NCH - 1:  # global key index 1984+p < L (r2)
                            p_r2 = pt[:, 512:768]
                            nc.gpsimd.affine_select(
                                out=p_r2, in_=p_r2, compare_op=is_ge, fill=0.0,
                                base=63, pattern=[[0, 256]], channel_multiplier=-1,
                            )
                        # P~ @ V (transposed): O'[d, l] with denom in row DH.
                        # Both heads share one PSUM bank (disjoint column halves;
                        # PE executes matmuls in program order, so hh=1's
                        # start=True bank-clear cannot interleave hh=0's group).
                        if hh == 0:
                            op = ps.tile(
                                [DH + 1, 512], f32, name=f"op{t}_{ch}", tag="o",
                                bufs=2,
                            )
                        for r in range(3):
                            vsl = Vs[2 * ch + r][
                                :, h * HEAD_STRIDE : (h + 1) * HEAD_STRIDE
                            ]
                            mm(
                                op[:, 256 * hh : 256 * (hh + 1)],
                                vsl,
                                pt[:, 256 * r : 256 * (r + 1)],
                                start=(r == 0),
                                stop=(r == 2),
                            )
                        if hh == 1:
                            otmp = pers.tile(
                                [DH + 1, 512], f32, name=f"otm{t}_{ch}", tag="otmp",
                                bufs=4,
                            )
                            nc.scalar.copy(out=otmp[:], in_=op[:])
                    # decoupled normalization for the head pair
                    rbp = ps.tile([DH, 512], f32, name=f"rbp{t}_{ch}", tag="big", bufs=2)
                    rr = pers.tile([1, 512], f32r, name=f"rr{t}_{ch}", tag="rr", bufs=2)
                    nc.vector.reciprocal(out=rr[:], in_=otmp[DH : DH + 1, :])
                    for hh in range(2):
                        mm(rbp[:, 256 * hh : 256 * (hh + 1)], ones1[:],
                           rr[:, 256 * hh : 256 * (hh + 1)], start=True, stop=True)
                    for hh in range(2):
                        nc.vector.tensor_mul(
                            out=OTc[t][64 * hh : 64 * (hh + 1), :],
                            in0=otmp[0:DH, 256 * hh : 256 * (hh + 1)],
                            in1=rbp[:, 256 * hh : 256 * (hh + 1)],
                        )
                # output projection for this chunk's two l-tiles
                for half in range(2):
                    lt = 2 * ch + half
                    yp = ps.tile([128, D], f32, name=f"yp{lt}", tag="big", bufs=2)
                    for k in range(NK):
                        mm(
                            yp[:],
                            OTc[k][:, half * 128 : (half + 1) * 128],
                            woutT[k][:],
                            start=(k == 0),
                            stop=(k == NK - 1),
                        )
                    ysb = pers.tile([128, D], f32, name=f"ysb{lt}", tag="ysb", bufs=2)
                    nc.vector.tensor_add(out=ysb[:], in0=yp[:], in1=boutb[:])
                    nc.sync.dma_start(out=y_d[lt * 128 : (lt + 1) * 128, :], in_=ysb[:])

    nc.compile()
    return nc


def get_nc():
    if "nc" not in _NC_CACHE:
        _NC_CACHE["nc"] = _build_nc()
    return _NC_CACHE["nc"]


def make_core_inputs(x, Wqkv, Wout, bout):
    """Host-side shard + layout prep: per-core transposed views."""
    x = np.asarray(x, dtype=np.float32)
    wqkvT = np.ascontiguousarray(np.asarray(Wqkv, dtype=np.float32).T)
    woutT = np.ascontiguousarray(np.asarray(Wout, dtype=np.float32).T)
    bout = np.ascontiguousarray(np.asarray(bout, dtype=np.float32))
    in_maps = []
    for b in range(B):
        in_maps.append(
            {
                "xT": np.ascontiguousarray(x[:, b, :].T),
                "wqkvT": wqkvT,
                "woutT": woutT,
                "bout": bout,
                "zeros_c": np.zeros((128, 640), dtype=np.float32),
                "ones_c": np.ones((128, 8), dtype=np.float32),
            }
        )
    return in_maps


def kernel(x, Wqkv, Wout, bout):
    from concourse.bass_utils import run_bass_kernel_spmd

    nc = get_nc()
    in_maps = make_core_inputs(x, Wqkv, Wout, bout)
    res = run_bass_kernel_spmd(nc, in_maps, core_ids=list(range(B)))
    out = np.empty((L, B, D), dtype=np.float32)
    for b in range(B):
        out[:, b, :] = res.results[b]["y"]
    return out


# revision 29
# speedup vs baseline: 23978.1592x; 23978.1592x over previous
"""Local-window (banded) multi-head attention on 8 Trainium2 NeuronCores.

Problem: x[L=2048, B=8, D=512], Wqkv[1536, 512], Wout[512, 512], bout[512].
  qkv = x @ Wqkv.T ; per-head banded attention (|i-j| <= 64, window 129);
  out = attn_out @ Wout.T + bout.

Sharding: batch B=8 across the 8 cores (data parallel). Each core runs the
full pipeline for one batch element. Inputs are pre-transposed host-side so
all device matmuls contract over the partition dimension:

  xT[d, l], WqkvT[d, c], WoutT[d', c] in SBUF; scores are computed
  TRANSPOSED (scoresT[m, l] = K @ Q^T) so that softmax normalization and
  the P@V contraction both happen along the partition (m) axis with zero
  on-chip transposes. The softmax denominator comes for free from an
  appended ones-column in V; normalization uses a tiny K=1 broadcast matmul.

Matmuls run in float32r (fp32 storage, fast PE path, N>=256).
"""

import os
import sys

import numpy as np

if "/opt/trn_rl_repo" not in sys.path:
    sys.path.insert(0, "/opt/trn_rl_repo")

L, B, D, H, DH = 2048, 8, 512, 8, 64
WIN, PAD = 129, 64
C3 = 3 * D  # 1536
NK = D // 128  # 4 contraction tiles
NLT = L // 128  # 16 l-tiles
NCH = L // 256  # 8 attention l-chunks of 256
HEAD_STRIDE = DH + 1  # 65: V columns per head incl. ones column

_NC_CACHE = {}


def _build_nc():
    from concourse import bacc, mybir, tile

    f32 = mybir.dt.float32
    f32r = mybir.dt.float32r
    Exp = mybir.ActivationFunctionType.Exp
    is_ge = mybir.AluOpType.is_ge

    nc = bacc.Bacc(None, target_bir_lowering=False)

    xT_d = nc.dram_tensor("xT", [D, L], f32r, kind="ExternalInput")
    wqkvT_d = nc.dram_tensor("wqkvT", [D, C3], f32r, kind="ExternalInput")
    woutT_d = nc.dram_tensor("woutT", [D, D], f32r, kind="ExternalInput")
    bout_d = nc.dram_tensor("bout", [D], f32, kind="ExternalInput")
    zeros_d = nc.dram_tensor("zeros_c", [128, 640], f32r, kind="ExternalInput")
    onesc_d = nc.dram_tensor("ones_c", [128, 8], f32r, kind="ExternalInput")
    y_d = nc.dram_tensor("y", [L, D], f32, kind="ExternalOutput")

    import concourse.bass as bass

    KTW = 64 + L + 64  # K^T cols: zero-pad both sides

    with tile.TileContext(nc) as tc, nc.allow_low_precision(
        reason="float32r tiles feed the PE fast path; accumulation stays fp32 in PSUM"
    ):
        with (
            tc.tile_pool(name="pers", bufs=1) as pers,
            tc.tile_pool(name="ps", bufs=1, space="PSUM") as ps,
        ):
            # ---- persistent SBUF tensors (everything stays resident) ----
            xT = [pers.tile([128, L], f32r, name=f"xT{k}", tag=f"xT{k}") for k in range(NK)]
            wqkvT = [
                pers.tile([128, C3], f32r, name=f"wqkvT{k}", tag=f"wqkvT{k}")
                for k in range(NK)
            ]
            woutT = [
                pers.tile([128, D], f32r, name=f"woutT{k}", tag=f"woutT{k}")
                for k in range(NK)
            ]
            boutb = pers.tile([128, D], f32, name="boutb", tag="boutb")
            ones1 = pers.tile([1, DH], f32r, name="ones1", tag="ones1")
            QT = [pers.tile([128, L], f32r, name=f"QT{t}", tag=f"QT{t}") for t in range(NK)]
            KT = [
                pers.tile([128, KTW], f32r, name=f"KT{t}", tag=f"KT{t}")
                for t in range(NK)
            ]
            Vs = [
                pers.tile([128, H * HEAD_STRIDE], f32r, name=f"Vs{j}", tag=f"Vs{j}")
                for j in range(NLT + 1)
            ]
            # per-chunk normalized O^T buffers (rotating, 2 l-tiles wide)
            OTc = [
                pers.tile([128, 256], f32r, name=f"OTc{t}", tag=f"OTc{t}", bufs=2)
                for t in range(NK)
            ]

            def mm(out, lhsT, rhs, start, stop):
                nc.tensor.matmul(out, lhsT, rhs, start=start, stop=stop)

            # ---- input DMAs: column-sliced + interleaved across both HWDGE
            # rings so the first projection groups unblock within ~3us ----
            for ch in range(4):
                cs = slice(ch * 512, (ch + 1) * 512)
                for k in range(NK):
                    eng = nc.sync if (k + ch) % 2 == 0 else nc.scalar
                    eng.dma_start(out=xT[k][:, cs], in_=xT_d[k * 128 : (k + 1) * 128, cs])
                # wqkvT thirds in Q, K, V priority order per round
                third = [0, D, 2 * D, None][ch]
                if third is not None:
                    ws = slice(third, third + 512)
                    for k in range(NK):
                        eng = nc.scalar if (k + ch) % 2 == 0 else nc.sync
                        eng.dma_start(
                            out=wqkvT[k][:, ws], in_=wqkvT_d[k * 128 : (k + 1) * 128, ws]
                        )
            for k in range(NK):
                nc.sync.dma_start(
                    out=woutT[k][:], in_=woutT_d[k * 128 : (k + 1) * 128, :]
                )
            bout_ap = bout_d[:]
            bout_bcast = bass.AP(
                tensor=bout_ap.tensor, offset=bout_ap.offset, ap=[[0, 128], [1, D]]
            )
            nc.gpsimd.dma_start(out=boutb[:], in_=bout_bcast)
            nc.gpsimd.dma_start(
                out=ones1[:], in_=onesc_d[0:DH, 0:1].rearrange("a b -> b a")
            )
            # zero K^T left pad and the out-of-range halves of the shifted V
            for t in range(NK):
                nc.sync.dma_start(out=KT[t][:, 0:64], in_=zeros_d[:, 0:64])
                nc.sync.dma_start(
                    out=KT[t][:, 64 + L : KTW], in_=zeros_d[:, 0:64]
                )
            nc.sync.dma_start(
                out=Vs[0][0:64, :], in_=zeros_d[0:64, 0 : H * HEAD_STRIDE]
            )
            nc.sync.dma_start(
                out=Vs[NLT][64:128, :], in_=zeros_d[0:64, 0 : H * HEAD_STRIDE]
            )
            # ones column for every head slot (softmax denom via PV matmul)
            for j in range(NLT + 1):
                vcol = Vs[j].rearrange("p (h e) -> p h e", e=HEAD_STRIDE)
                nc.gpsimd.dma_start(
                    out=vcol[:, :, DH : DH + 1],
                    in_=onesc_d[:].rearrange("p (h e) -> p h e", e=1),
                )

            # ---- phase B: projections, interleaved so attention unblocks
            # early: Q/K chunk round first, then a slice of V tiles ----
            def b1_vproj(lts):
                for lt in lts:
                    vp = ps.tile([128, D], f32, name=f"vp{lt}", tag="big", bufs=2)
                    for k in range(NK):
                        mm(
                            vp[:],
                            xT[k][:, lt * 128 : (lt + 1) * 128],
                            wqkvT[k][:, 2 * D : 3 * D],
                            start=(k == 0),
                            stop=(k == NK - 1),
                        )
                    src_v = vp.rearrange("p (h e) -> p h e", e=DH)
                    dlo = Vs[lt][64:128, :].rearrange("p (h e) -> p h e", e=HEAD_STRIDE)
                    dhi = Vs[lt + 1][0:64, :].rearrange(
                        "p (h e) -> p h e", e=HEAD_STRIDE
                    )
                    nc.scalar.copy(out=dlo[:, :, 0:DH], in_=src_v[0:64])
                    nc.vector.tensor_copy(out=dhi[:, :, 0:DH], in_=src_v[64:128])

            for ch in range(4):  # l-chunks of 512
                for t in range(NK):
                    for which in range(2):  # 0 -> Q tile t, 1 -> K tile t
                        c0 = which * D + t * 128
                        qp = ps.tile(
                            [128, 512], f32, name=f"qp{t}_{which}_{ch}",
                            tag="big", bufs=2,
                        )
                        for k in range(NK):
                            mm(
                                qp[:],
                                wqkvT[k][:, c0 : c0 + 128],
                                xT[k][:, ch * 512 : (ch + 1) * 512],
                                start=(k == 0),
                                stop=(k == NK - 1),
                            )
                        if which == 0:
                            dest = QT[t][:, ch * 512 : (ch + 1) * 512]
                        else:
                            dest = KT[t][:, 64 + ch * 512 : 64 + (ch + 1) * 512]
                        nc.vector.tensor_copy(out=dest, in_=qp[:])
                b1_vproj(range(4 * ch, 4 * ch + 4))

            # ---- phase C+D: banded attention + fused output projection ----
            for ch in range(NCH):
                for t in range(NK):
                    otc = OTc[t] if True else None
                    otmp = None
                    for hh in range(2):
                        h = 2 * t + hh
                        p0 = hh * 64
                        qsl = QT[t][p0 : p0 + 64, ch * 256 : (ch + 1) * 256]
                        # fused scores psum: 3 m-tiles side by side (2 banks)
                        scp = ps.tile(
                            [128, 768], f32, name=f"sc{h}_{ch}", tag="sc", bufs=2
                        )
                        for r in range(3):
                            kcol = 256 * ch + 128 * r  # into padded KT columns
                            mm(
                                scp[:, 256 * r : 256 * (r + 1)],
                                KT[t][p0 : p0 + 64, kcol : kcol + 128],
                                qsl,
                                start=True,
                                stop=True,
                            )
                        pt = wk_tile = pers.tile(
                            [128, 768], f32r, name=f"pt{h}_{ch}", tag="pt", bufs=4
                        )
                        nc.scalar.activation(
                            out=pt[:], in_=scp[:], func=Exp, scale=0.125
                        )
                        # band mask per m-tile r: keep iff 0 <= (128r + p) - f <= 128
                        # fused as two 2-block selects over the 768-wide tile
                        pAB = pt[:, 0:512].rearrange("p (b f) -> p b f", f=256)
                        pBC = pt[:, 256:768].rearrange("p (b f) -> p b f", f=256)
                        nc.gpsimd.affine_select(
                            out=pAB, in_=pAB, compare_op=is_ge, fill=0.0,
                            base=0, pattern=[[128, 2], [-1, 256]],
                            channel_multiplier=1,
                        )
                        nc.gpsimd.affine_select(
                            out=pBC, in_=pBC, compare_op=is_ge, fill=0.0,
                            base=0, pattern=[[-128, 2], [1, 256]],
                            channel_multiplier=-1,
                        )
                        if ch == 0:  # global key index p-64 must be >= 0 (r0)
                            p_r0 = pt[:, 0:256]
                            nc.gpsimd.affine_select(
                                out=p_r0, in_=p_r0, compare_op=is_ge, fill=0.0,
                                base=-64, pattern=[[0, 256]], channel_multiplier=1,
                            )
                        if ch == NCH - 1:  # global key index 1984+p < L (r2)
                            p_r2 = pt[:, 512:768]
                            nc.gpsimd.affine_select(
                                out=p_r2, in_=p_r2, compare_op=is_ge, fill=0.0,
                                base=63, pattern=[[0, 256]], channel_multiplier=-1,
                            )
                        # P~ @ V (transposed): O'[d, l] with denom in row DH.
                        # Both heads share one PSUM bank (disjoint column halves;
                        # PE executes matmuls in program order, so hh=1's
                        # start=True bank-clear cannot interleave hh=0's group).
                        if hh == 0:
                            op = ps.tile(
                                [DH + 1, 512], f32, name=f"op{t}_{ch}", tag="o",
                                bufs=2,
                            )
                        for r in range(3):
                            vsl = Vs[2 * ch + r][
                                :, h * HEAD_STRIDE : (h + 1) * HEAD_STRIDE
                            ]
                            mm(
                                op[:, 256 * hh : 256 * (hh + 1)],
                                vsl,
                                pt[:, 256 * r : 256 * (r + 1)],
                                start=(r == 0),
                                stop=(r == 2),
                            )
                        if hh == 1:
                            otmp = pers.tile(
                                [DH + 1, 512], f32, name=f"otm{t}_{ch}", tag="otmp",
                                bufs=4,
                            )
                            nc.scalar.copy(out=otmp[:], in_=op[:])
                    # decoupled normalization for the head pair
                    rbp = ps.tile([DH, 512], f32, name=f"rbp{t}_{ch}", tag="big", bufs=2)
                    rr = pers.tile([1, 512], f32r, name=f"rr{t}_{ch}", tag="rr", bufs=2)
                    nc.vector.reciprocal(out=rr[:], in_=otmp[DH : DH + 1, :])
                    for hh in range(2):
                        mm(rbp[:, 256 * hh : 256 * (hh + 1)], ones1[:],
                           rr[:, 256 * hh : 256 * (hh + 1)], start=True, stop=True)
                    for hh in range(2):
                        nc.vector.tensor_mul(
                            out=OTc[t][64 * hh : 64 * (hh + 1), :],
                            in0=otmp[0:DH, 256 * hh : 256 * (hh + 1)],
                            in1=rbp[:, 256 * hh : 256 * (hh + 1)],
                        )
                # output projection for this chunk's two l-tiles
                for half in range(2):
                    lt = 2 * ch + half
                    yp = ps.tile([128, D], f32, name=f"yp{lt}", tag="big", bufs=2)
                    for k in range(NK):
                        mm(
                            yp[:],
                            OTc[k][:, half * 128 : (half + 1) * 128],
                            woutT[k][:],
                            start=(k == 0),
                            stop=(k == NK - 1),
                        )
                    ysb = pers.tile([128, D], f32, name=f"ysb{lt}", tag="ysb", bufs=2)
                    nc.vector.tensor_add(out=ysb[:], in0=yp[:], in1=boutb[:])
                    nc.sync.dma_start(out=y_d[lt * 128 : (lt + 1) * 128, :], in_=ysb[:])

    nc.compile()
    return nc


def get_nc():
    if "nc" not in _NC_CACHE:
        _NC_CACHE["nc"] = _build_nc()
    return _NC_CACHE["nc"]


def make_core_inputs(x, Wqkv, Wout, bout):
    """Host-side shard + layout prep: per-core transposed views."""
    x = np.asarray(x, dtype=np.float32)
    wqkvT = np.ascontiguousarray(np.asarray(Wqkv, dtype=np.float32).T)
    woutT = np.ascontiguousarray(np.asarray(Wout, dtype=np.float32).T)
    bout = np.ascontiguousarray(np.asarray(bout, dtype=np.float32))
    in_maps = []
    for b in range(B):
        in_maps.append(
            {
                "xT": np.ascontiguousarray(x[:, b, :].T),
                "wqkvT": wqkvT,
                "woutT": woutT,
                "bout": bout,
                "zeros_c": np.zeros((128, 640), dtype=np.float32),
                "ones_c": np.ones((128, 8), dtype=np.float32),
            }
        )
    return in_maps


def kernel(x, Wqkv, Wout, bout):
    from concourse.bass_utils import run_bass_kernel_spmd

    nc = get_nc()
    in_maps = make_core_inputs(x, Wqkv, Wout, bout)
    res = run_bass_kernel_spmd(nc, in_maps, core_ids=list(range(B)))
    out = np.empty((L, B, D), dtype=np.float32)
    for b in range(B):
        out[:, b, :] = res.results[b]["y"]
    return out
